# revision 20
# baseline (speedup 1.0000x reference)
"""Akima spline interpolation kernel for Trainium2 (8 NeuronCores, data parallel).

Strategy (v2):
  - Host evaluates the exact float64 Akima spline at the edges of 2049
    node-centered segments (idx = round(2048*x)) and builds a continuous
    piecewise-linear approximation: out = f[j] + b[j]*(x - j/2048).
    f (center value) and b (slope) are stored as an f16 pair packed into
    one uint32 word per segment -> a single 2052-word replicated table.
  - Device (NKI per core):
      rbig = x + 6144.0        (Scalar; rounds x to the 2^-11 grid: the
                                binade [4096,8192) has ulp 2^-11)
      idxf = rbig - 6144.0     (Scalar; = j*2^-11 exactly)
      idx  = u32(2048*rbig - 12582912)   (Scalar; = j exactly, fused affine)
      w    = gather_flattened(table, idx)   (GpSimd; one word per element)
      vv   = x - idxf          (DVE; exact, |vv| <= 2^-12)
      m    = vv * w.f16[odd]   (DVE; slope)
      out  = m + w.f16[even]   (DVE; -> f16 output, halves store traffic)
  - Sharding: pure data parallel on the leading dim (4 of 32 planes per
    core); the 8.2KB table is replicated to all partitions of every core.
  - Accuracy: rel_l2 ~ 8.5e-4 (PWL-2048 + f16 table + f16 out), vs the
    2e-2 gate.
"""
import base64
import json
import sys

import numpy as np

if "/opt/trn_rl_repo" not in sys.path:
    sys.path.insert(0, "/opt/trn_rl_repo")

import os

NODES = 256
N_CORES = 8
ROWS = 128
COLS = 4 * 1024 * 1024 // ROWS  # per-core shard [128, 32768]
F_TILE = 2048
NSEG = int(os.environ.get("AKIMA_NSEG", "1024"))  # idx = round(NSEG*x) in [0, NSEG]
PACK = os.environ.get("AKIMA_PACK", "f16pair")   # f16pair (u32) | i8pair (u16)
TAB_N = NSEG + 1 + int(os.environ.get("AKIMA_PAD", "31"))  # pad: staged-tail guard
MAGIC = float(np.float32(1.5 * 2.0 ** 23))       # 12582912.0
BIAS_GRID = float(np.float32(1.5 * 2.0 ** 23 / NSEG))  # rounds x to the 1/NSEG grid
BIAS_IDX = float(np.float32(-1.5 * 2.0 ** 23))   # -12582912.0
OUT_MODE = os.environ.get("AKIMA_OUT", "f16split")  # f16direct | f16split | f32

# ----------------------------------------------------------------------------
# Host-side table construction (float64 exact spline -> PWL f16 pair table)
# ----------------------------------------------------------------------------


def _akima_slopes_f64(value):
    h = 1.0 / (NODES - 1)
    v = value.astype(np.float64)
    m = (v[1:] - v[:-1]) / h
    m_m1 = 2.0 * m[0] - m[1]
    m_m2 = 2.0 * m_m1 - m[0]
    m_p1 = 2.0 * m[-1] - m[-2]
    m_p2 = 2.0 * m_p1 - m[-1]
    me = np.concatenate([[m_m2, m_m1], m, [m_p1, m_p2]])
    w1 = np.abs(me[3:] - me[2:-1])
    w2 = np.abs(me[1:-2] - me[:-3])
    mi_1 = me[1:-2]
    mi = me[2:-1]
    denom = w1 + w2
    safe = np.where(denom > 0, denom, 1.0)
    return np.where(denom > 0, (w1 * mi_1 + w2 * mi) / safe, 0.5 * (mi_1 + mi))


def _spline_eval_f64(xq, value):
    h = 1.0 / (NODES - 1)
    s = _akima_slopes_f64(value)
    v = value.astype(np.float64)
    t = np.clip(xq, 0.0, 1.0) / h
    idx = np.clip(np.floor(t).astype(np.int64), 0, NODES - 2)
    u = t - idx
    v0, v1 = v[idx], v[idx + 1]
    s0, s1 = s[idx], s[idx + 1]
    u2 = u * u
    u3 = u2 * u
    return ((2 * u3 - 3 * u2 + 1) * v0 + (u3 - 2 * u2 + u) * h * s0
            + (-2 * u3 + 3 * u2) * v1 + (u3 - u2) * h * s1)


def _build_table(value):
    """Returns (table_words, sf, off). f16pair: u32 words, sf/off unused.
    i8pair: u16 words (low byte f_i8, high byte b_i8), out = (vv*b8 + f8)*sf + off."""
    h = 1.0 / NSEG
    edges = (np.arange(NSEG + 2) - 0.5) * h
    edges = np.clip(edges, 0.0, 1.0)
    fe = _spline_eval_f64(edges, value)
    e0, e1 = edges[:-1], edges[1:]
    bx = (fe[1:] - fe[:-1]) / (e1 - e0)
    xc = np.arange(NSEG + 1) * h
    fc = fe[:-1] + bx * (xc - e0)
    if PACK == "f16pair":
        f16 = fc.astype(np.float16)
        b16 = bx.astype(np.float16)
        word = (f16.view(np.uint16).astype(np.uint32)
                | (b16.view(np.uint16).astype(np.uint32) << 16))
        out = np.empty(TAB_N, dtype=np.uint32)
        out[:NSEG + 1] = word
        out[NSEG + 1:] = word[-1]
        return out, 1.0, 0.0
    # i8pair: slopes in t units, shared affine (sf, off)
    bt = bx * h
    off = (fc.max() + fc.min()) / 2.0
    sf = (fc.max() - fc.min()) / 254.0
    f8 = np.clip(np.round((fc - off) / sf), -127, 127).astype(np.int8)
    b8 = np.clip(np.round(bt / sf), -127, 127).astype(np.int8)
    word = (f8.view(np.uint8).astype(np.uint16)
            | (b8.view(np.uint8).astype(np.uint16) << 8))
    out = np.empty(TAB_N, dtype=np.uint16)
    out[:NSEG + 1] = word
    out[NSEG + 1:] = word[-1]
    return out, float(sf), float(off)


# ----------------------------------------------------------------------------
# NKI kernel
# ----------------------------------------------------------------------------


def _make_nki_kernel(sf, off):
    import neuronxcc.nki.language as nl
    import neuronxcc.nki.isa as nisa

    n_tiles = COLS // F_TILE

    def akima_kernel(inputs):
        x, table = inputs[0], inputs[1]
        out_dt = nl.float32 if OUT_MODE == "f32" else nl.float16
        out = nl.ndarray(shape=[ROWS, COLS], dtype=out_dt, buffer=nl.shared_hbm)
        tab_sb = nl.load(table)
        i_p = nl.arange(ROWS)[:, None]
        i_f = nl.arange(F_TILE)[None, :]
        bias_grid = nisa.memset((ROWS, 1), BIAS_GRID, nl.float32)
        neg_bias_grid = nisa.memset((ROWS, 1), -BIAS_GRID, nl.float32)
        bias_idx = nisa.memset((ROWS, 1), BIAS_IDX, nl.float32)
        bias_magic = nisa.memset((ROWS, 1), MAGIC, nl.float32)
        bias_off = nisa.memset((ROWS, 1), off, nl.float32)

        # Explicit ping-pong SBUF buffers to avoid WAR serialization.
        bufs = []
        for _pp in nl.static_range(2):
            bufs.append(dict(
                rbig=nl.ndarray(shape=[ROWS, F_TILE], dtype=nl.float32, buffer=nl.sbuf),
                idxf=nl.ndarray(shape=[ROWS, F_TILE], dtype=nl.float32, buffer=nl.sbuf),
                idx=nl.ndarray(shape=[ROWS, F_TILE], dtype=nl.uint32, buffer=nl.sbuf),
                ts1=nl.ndarray(shape=[ROWS, F_TILE], dtype=nl.float32, buffer=nl.sbuf),
                w=nl.ndarray(shape=[ROWS, F_TILE],
                             dtype=(nl.uint32 if PACK == "f16pair" else nl.uint16),
                             buffer=nl.sbuf),
                vv=nl.ndarray(shape=[ROWS, F_TILE], dtype=nl.float32, buffer=nl.sbuf),
                m=nl.ndarray(shape=[ROWS, F_TILE], dtype=nl.float32, buffer=nl.sbuf),
            ))

        for t in nl.static_range(n_tiles):
            B = bufs[t % 2]
            sl = slice(t * F_TILE, (t + 1) * F_TILE)
            x_sb = nl.load(x[:, sl])
            if PACK == "f16pair":
                # x-units: rbig = x + BIAS_GRID; idxf = j/NSEG; vv = x - idxf
                B['rbig'][i_p, i_f] = nisa.activation(
                    np.copy, x_sb, bias=bias_grid)
                B['idxf'][i_p, i_f] = nisa.activation(
                    np.copy, B['rbig'][i_p, i_f], bias=neg_bias_grid)
                B['idx'][i_p, i_f] = nisa.activation(
                    np.copy, B['rbig'][i_p, i_f], bias=bias_idx,
                    scale=float(NSEG), dtype=nl.uint32)
                B['w'][i_p, i_f] = nl.gather_flattened(
                    data=tab_sb, indices=B['idx'][i_p, i_f])
                B['vv'][i_p, i_f] = nisa.tensor_tensor(
                    x_sb, B['idxf'][i_p, i_f], np.subtract)
                w16 = B['w'].view(nl.float16)
                B['m'][i_p, i_f] = nisa.tensor_tensor(
                    B['vv'][i_p, i_f], w16[i_p, i_f * 2 + 1], np.multiply,
                    dtype=nl.float32)
                if OUT_MODE == "f16direct":
                    r = nisa.tensor_tensor(
                        B['m'][i_p, i_f], w16[i_p, i_f * 2], np.add,
                        dtype=nl.float16)
                elif OUT_MODE == "f16split":
                    p32 = nisa.tensor_tensor(
                        B['m'][i_p, i_f], w16[i_p, i_f * 2], np.add,
                        dtype=nl.float32)
                    r = nisa.tensor_copy(p32, dtype=nl.float16)
                else:
                    r = nisa.tensor_tensor(
                        B['m'][i_p, i_f], w16[i_p, i_f * 2], np.add,
                        dtype=nl.float32)
            else:
                # i8pair, t-units: rbig = NSEG*x + MAGIC; idxf = j; vv = t - j
                B['rbig'][i_p, i_f] = nisa.activation(
                    np.copy, x_sb, scale=float(NSEG), bias=bias_magic)
                B['idxf'][i_p, i_f] = nisa.activation(
                    np.copy, B['rbig'][i_p, i_f], bias=bias_idx)
                B['idx'][i_p, i_f] = nisa.activation(
                    np.copy, B['rbig'][i_p, i_f], bias=bias_idx,
                    dtype=nl.uint32)
                B['ts1'][i_p, i_f] = nisa.tensor_scalar(
                    x_sb, np.multiply, float(NSEG),
                    engine=nisa.vector_engine)
                B['w'][i_p, i_f] = nl.gather_flattened(
                    data=tab_sb, indices=B['idx'][i_p, i_f])
                B['vv'][i_p, i_f] = nisa.tensor_tensor(
                    B['ts1'][i_p, i_f], B['idxf'][i_p, i_f], np.subtract)
                w8 = B['w'].view(nl.int8)
                B['m'][i_p, i_f] = nisa.tensor_tensor(
                    B['vv'][i_p, i_f], w8[i_p, i_f * 2 + 1], np.multiply,
                    dtype=nl.float32)
                p32 = nisa.tensor_tensor(
                    B['m'][i_p, i_f], w8[i_p, i_f * 2], np.add,
                    dtype=nl.float32)
                r = nisa.activation(
                    np.copy, p32, scale=float(sf), bias=bias_off,
                    dtype=(nl.float32 if OUT_MODE == "f32" else nl.float16))
            nl.store(out[:, sl], r)
        return [out]

    return akima_kernel


# ----------------------------------------------------------------------------
# jax integration (AwsNeuronCustomNativeKernel custom call, SPMD over 8 cores)
# ----------------------------------------------------------------------------

_EXEC_CACHE = {}


def _build_executor(sf, off):
    key = (sf, off)
    if key in _EXEC_CACHE:
        return _EXEC_CACHE[key]

    import jax
    from jax.interpreters import mlir
    from jax._src.interpreters.mlir import custom_call as _mlir_custom_call
    from jax.sharding import Mesh, PartitionSpec
    from jax.experimental.shard_map import shard_map
    from concourse.nki import raw_nki
    from concourse.bass2jax import install_neuronx_cc_hook

    install_neuronx_cc_hook()

    nki_func = _make_nki_kernel(sf, off)

    prim = jax.extend.core.Primitive(f"akima_exec_v2_{len(_EXEC_CACHE)}")
    prim.multiple_results = True

    out_np = np.float32 if OUT_MODE == "f32" else np.float16

    @prim.def_abstract_eval
    def _abs(*_, **__):
        return (jax.core.ShapedArray((ROWS, COLS), out_np),)

    def _layouts(shapes):
        return [list(reversed(range(len(s)))) for s in shapes]

    def _lowering(ctx, *in_nodes):
        from neuronxcc.starfish.penguin.ir.NativeKernel import KERNEL_VERSION

        result_types = [mlir.aval_to_ir_type(a) for a in ctx.avals_out]
        code = raw_nki(nki_func)(list(ctx.avals_in))
        config = {
            "kernel_version": KERNEL_VERSION,
            "func_literal": code.serialize_ir_string("akima_kernel_ir"),
            "grid": [],
            "func_name": "akima_kernel",
            "has_collectives": False,
            "mac_count": 0,
            "tiled": False,
        }
        dumped = base64.b64encode(json.dumps(config).encode()).decode()
        return _mlir_custom_call(
            "AwsNeuronCustomNativeKernel",
            operands=list(in_nodes),
            result_types=result_types,
            operand_layouts=_layouts(a.shape for a in ctx.avals_in),
            result_layouts=_layouts(a.shape for a in ctx.avals_out),
            backend_config=dumped,
        ).results

    mlir.register_lowering(prim, _lowering, platform="neuron")

    devices = jax.devices()[:N_CORES]
    mesh = Mesh(np.asarray(devices), ("core",))

    def _body(x_shard, tab_shard):
        return prim.bind(x_shard, tab_shard)[0]

    sharded = jax.jit(shard_map(
        _body, mesh=mesh,
        in_specs=(PartitionSpec("core"), PartitionSpec("core")),
        out_specs=PartitionSpec("core"),
        check_rep=False,
    ))

    _EXEC_CACHE[key] = sharded
    return sharded


# ----------------------------------------------------------------------------
# Public entry point
# ----------------------------------------------------------------------------


def kernel(input: np.ndarray, value: np.ndarray) -> np.ndarray:
    input = np.ascontiguousarray(np.asarray(input, dtype=np.float32))
    value = np.asarray(value, dtype=np.float32)
    assert input.shape == (32, 1024, 1024), input.shape

    word, sf, off = _build_table(value)
    table = np.broadcast_to(word, (ROWS, TAB_N)).copy()

    sharded = _build_executor(sf, off)

    # shard on the leading dim: core i gets planes [4i, 4i+4)
    x_global = input.reshape(N_CORES * ROWS, COLS)
    tab_global = np.tile(table, (N_CORES, 1))

    out = sharded(x_global, tab_global)
    return np.asarray(out).astype(np.float32).reshape(32, 1024, 1024)


if __name__ == "__main__":
    inp = np.load("cache/input.npy")
    val = np.load("cache/value.npy")
    out = kernel(input=inp, value=val)
    exp = np.load("cache/expected.npy")
    err = out.astype(np.float64) - exp.astype(np.float64)
    print("rel_l2:", np.linalg.norm(err) / np.linalg.norm(exp))
